# revision 10
# baseline (speedup 1.0000x reference)
"""3-layer GCN forward (GCNConv x3 + log_softmax) on 8 Trainium2 cores.

v2: dma_gather-based aggregation (vs per-step indirect DMA).

Key ideas (shapes hardcoded for N=100000, Cin=Ch=128, Cout=47, 8 cores):
  A_hat = D^-1/2 (A+I) D^-1/2 fixed across layers:
      out = dinv_dst * segsum_dst( dinv_src * (H @ W) ) + b
  All feature math in fp16 (weights, Z exchange, gathers) with fp32 PSUM
  accumulation; rel-err budget is 2e-2.

  Aggregation uses nc.gpsimd.dma_gather (batched token gather): one
  instruction gathers up to ~6K rows. Its indices are int16 (<32768), so
  the 100352-row Z table is addressed via 4 residue classes mod 4:
  class m = rows with padded-global position % 4 == m, reached with
  elem_step=512 elements (1024B row stride) and base offset m*128 elems;
  idx = position // 4 <= 25087.

  Host side chooses the node permutation: degree-sorted round-robin deal
  across cores (aligns group degree profiles), then a greedy residue
  assignment balancing each destination row's in-messages across the 4
  classes (minimizes the per-(group,class) column padding), then nodes
  are placed within their 128-row group at slots == residue (mod 4).

  Column grid: per (group g, class m), cmax[g,m] = max over cores and
  over the group's 128 rows of the per-row class-m message count.
  Columns laid out block-major (blocks of 4 groups), class runs
  contiguous per (block, class) -> one dma_gather per (block, class).
  Accumulation: identity-matmul into a per-group PSUM bank with 4 lanes
  (psum [128, 4*128]), lane-reduced on DVE, then dinv_dst scale + bias
  + relu (or log_softmax on the last layer).

z row space: node (core k, local r) lives at padded-global row
k*12544 + r; rows [12500,12544) of each core block are zero (dinv=0
forces Z pads to 0); pad gather slots point at rows 12500+m (idx 3125).
"""
import numpy as np

NCORES = 8
N = 100000
NBLK = 12500
NPAD = 12544            # 98 * 128
NGRP = NPAD // 128      # 98
NALL = NCORES * NPAD    # 100352
C = 128
COUT = 47
BLK = 4                 # groups per gather block
ZPAD16 = (NCORES * NPAD // 8) and 3125  # (12500+m)//4 for m in 0..3


def _residue_greedy(src, dst, dstp, core_of, grp_of, d_g, rank):
    """Assign each node a residue class 0..3 balancing per-dst-row,
    per-class in-message counts against group thresholds ceil(d_g/4)."""
    order_src = np.argsort(src, kind="stable")
    dst_by_src = dstp[order_src]
    starts = np.zeros(N + 1, np.int64)
    cnt_src = np.bincount(src, minlength=N)
    np.cumsum(cnt_src, out=starts[1:])

    grp_of_pos = (np.arange(N) % NBLK) // 128
    T_pos = ((d_g + 3) // 4)[grp_of_pos].astype(np.int32)

    c = np.zeros((N, 4), np.int32)
    capq = np.full((NCORES, NGRP, 4), 32, np.int32)
    capq[:, NGRP - 1, :] = 21      # group 97 has 84 real rows
    res = np.full(N, -1, np.int8)
    BIG = 1 << 40
    for v in rank:
        s, e = starts[v], starts[v + 1]
        rows = dst_by_src[s:e]
        k, g = core_of[v], grp_of[v]
        cr = c[rows]
        over = cr + 1 - T_pos[rows][:, None]
        pen = np.where(over > 0, 1 << (4 * np.minimum(over, 9)), 0).astype(
            np.int64)
        sc = pen.sum(axis=0) + cr.sum(axis=0)
        sc = np.where(capq[k, g] > 0, sc, BIG)
        m = int(np.argmin(sc))
        res[v] = m
        capq[k, g, m] -= 1
        c[rows, m] += 1
    return res


def _preprocess(x, edge_index, W1, b1, W2, b2, W3, b3):
    import ml_dtypes
    bf = np.float16
    x = np.asarray(x, np.float32)
    ei = np.asarray(edge_index)
    loop = np.arange(N, dtype=np.int64)
    src = np.concatenate([ei[0], loop]).astype(np.int64)
    dst = np.concatenate([ei[1], loop]).astype(np.int64)

    deg = np.bincount(dst, minlength=N).astype(np.float32)
    dinv = 1.0 / np.sqrt(np.maximum(deg, 1.0))

    # degree-ranked round-robin deal across cores
    rank = np.argsort(-deg, kind="stable")
    perm = np.empty(N, np.int64)
    for k in range(NCORES):
        perm[k * NBLK:(k + 1) * NBLK] = rank[k::NCORES]
    inv = np.empty(N, np.int64)
    inv[perm] = np.arange(N)

    # unchunked per-group max degree (threshold source)
    dstp0 = inv[dst]
    cnt_tot = np.bincount(dstp0, minlength=N)
    cp = np.zeros((NCORES, NPAD), np.int64)
    cp[:, :NBLK] = cnt_tot.reshape(NCORES, NBLK)
    d_g = cp.reshape(NCORES, NGRP, 128).max(axis=2).max(axis=0)

    core_of = inv // NBLK
    grp_of = (inv % NBLK) // 128
    res = _residue_greedy(src, dst, dstp0, core_of, grp_of, d_g, rank)

    # re-place nodes within their group at slots == residue (mod 4)
    pos = np.arange(N)
    core_p = pos // NBLK
    r_p = pos % NBLK
    g_p = r_p // 128
    node_at = perm
    res_p = res[node_at].astype(np.int64)
    key2 = (core_p * NGRP + g_p) * 4 + res_p
    order2 = np.argsort(key2, kind="stable")
    cnt2 = np.bincount(key2, minlength=NCORES * NGRP * 4)
    cs2 = np.zeros(NCORES * NGRP * 4 + 1, np.int64)
    np.cumsum(cnt2, out=cs2[1:])
    jj = np.arange(N) - cs2[key2[order2]]
    slot = res_p[order2] + 4 * jj
    newpos = core_p[order2] * NBLK + g_p[order2] * 128 + slot
    perm2 = np.empty(N, np.int64)
    perm2[newpos] = node_at[order2]
    inv2 = np.empty(N, np.int64)
    inv2[perm2] = np.arange(N)

    # message coordinates under perm2
    srcp = inv2[src]
    dstp = inv2[dst]
    ksrc = srcp // NBLK
    srcg = ksrc * NPAD + (srcp - ksrc * NBLK)      # padded-global
    m_arr = (srcg % 4).astype(np.int64)
    idx16 = (srcg // 4).astype(np.int64)
    ecore = dstp // NBLK
    rloc = dstp - ecore * NBLK
    g_arr = rloc // 128

    # per (core,row,class) counts -> shared cmax[g,m]
    keyc = (ecore * NBLK + rloc) * 4 + m_arr
    cntc = np.bincount(keyc, minlength=NCORES * NBLK * 4)
    cpc = np.zeros((NCORES, NPAD, 4), np.int64)
    cpc[:, :NBLK, :] = cntc.reshape(NCORES, NBLK, 4)
    cmax = cpc.reshape(NCORES, NGRP, 128, 4).max(axis=2).max(axis=0)
    tot = cmax.sum(axis=1)
    bump = np.maximum(0, 4 - tot)
    cmax[:, 0] += bump                              # lanes need >= 4 cols

    # block-major class-run column layout
    blocks = [list(range(b, min(NGRP, b + BLK))) for b in range(0, NGRP, BLK)]
    col_off = np.zeros((NGRP, 4), np.int64)
    posc = 0
    binfo = []
    for blkg in blocks:
        runs = []
        for m in range(4):
            c0 = posc
            for g in blkg:
                col_off[g, m] = posc
                posc += cmax[g, m]
            runs.append((int(c0), int(posc - c0)))
        binfo.append((blkg, runs))
    n_cols = int(posc)
    cwmax = max(cw for _, runs in binfo for _, cw in runs)

    # gather tables [NCORES, 128, n_cols] int16
    order = np.lexsort((srcg, m_arr, rloc, ecore))
    e_s = ecore[order]
    r_s = rloc[order]
    m_s = m_arr[order]
    i_s = idx16[order]
    keys = (e_s * NBLK + r_s) * 4 + m_s
    css = np.zeros(NCORES * NBLK * 4 + 1, np.int64)
    np.cumsum(cntc, out=css[1:])
    j_s = np.arange(len(keys)) - css[keys]
    col_s = col_off[r_s // 128, m_s] + j_s
    tbl = np.full((NCORES, 128, n_cols), ZPAD16, np.int16)
    tbl[e_s, r_s % 128, col_s] = i_s.astype(np.int16)

    # wrap: flat i = col*128 + p lives at [i%16, i//16]
    idxarr = np.ascontiguousarray(
        tbl.transpose(0, 2, 1).reshape(NCORES, n_cols, 8, 16)
        .transpose(0, 3, 1, 2).reshape(NCORES, 16, n_cols * 8))

    dinv2 = dinv[perm2]
    dv = dinv2.reshape(NCORES, NBLK)
    dinv_loc = np.zeros((NCORES, 128, NGRP), np.float32)
    for k in range(NCORES):
        full = np.zeros(NPAD, np.float32)
        full[:NBLK] = dv[k]
        dinv_loc[k] = full.reshape(NGRP, 128).T

    xp = x[perm2]
    xT = np.zeros((NCORES, C, NPAD), bf)
    for k in range(NCORES):
        xT[k, :, :NBLK] = xp[k * NBLK:(k + 1) * NBLK].T.astype(bf)

    w3p = np.zeros((C, C), np.float32)
    w3p[:, :COUT] = np.asarray(W3, np.float32)
    Ws = [np.asarray(W1, np.float32).astype(bf),
          np.asarray(W2, np.float32).astype(bf),
          w3p.astype(bf)]
    bbs = [np.tile(np.asarray(b, np.float32)[None, :], (128, 1))
           for b in (b1, b2, b3)]

    in_maps = []
    for k in range(NCORES):
        in_maps.append({
            "xT": np.ascontiguousarray(xT[k]),
            "gidx": np.ascontiguousarray(idxarr[k]),
            "dinv": np.ascontiguousarray(dinv_loc[k]),
            "w1": Ws[0], "w2": Ws[1], "w3": Ws[2],
            "bb1": np.ascontiguousarray(bbs[0]),
            "bb2": np.ascontiguousarray(bbs[1]),
            "bb3": np.ascontiguousarray(bbs[2]),
        })
    meta = {
        "cmax": cmax.astype(int).tolist(),
        "col_off": col_off.astype(int).tolist(),
        "binfo": binfo,
        "n_cols": n_cols,
        "cwmax": int(cwmax),
    }
    return in_maps, meta, perm2


def _build(meta):
    from concourse import bacc, bass, mybir, tile
    from concourse.masks import make_identity
    from concourse import library_config
    f32 = mybir.dt.float32
    bf16 = mybir.dt.float16
    i16 = mybir.dt.int16
    cmax = meta["cmax"]
    col_off = meta["col_off"]
    binfo = meta["binfo"]
    n_cols = meta["n_cols"]
    couts = [C, C, COUT]

    nc = bacc.Bacc("TRN2", target_bir_lowering=False, debug=False,
                   num_devices=NCORES)
    xT_d = nc.dram_tensor("xT", [C, NPAD], bf16, kind="ExternalInput")
    gidx = nc.dram_tensor("gidx", [16, n_cols * 8], i16,
                          kind="ExternalInput")
    dinv = nc.dram_tensor("dinv", [128, NGRP], f32, kind="ExternalInput")
    w_in = [nc.dram_tensor(f"w{l+1}", [C, C], bf16, kind="ExternalInput")
            for l in range(3)]
    bb_in = [nc.dram_tensor(f"bb{l+1}", [128, couts[l]], f32,
                            kind="ExternalInput") for l in range(3)]
    out_d = nc.dram_tensor("out", [NPAD, COUT], f32, kind="ExternalOutput")

    zs = nc.dram_tensor("zs", [NPAD, C], bf16)
    zf = [nc.dram_tensor(f"zf{l}", [NALL, C], bf16, addr_space="Shared")
          for l in range(3)]

    with tile.TileContext(nc) as tc:
        with tc.tile_pool(name="const", bufs=1) as cpool, \
             tc.tile_pool(name="hbuf", bufs=1) as hpool, \
             tc.tile_pool(name="gath", bufs=4) as gpool, \
             tc.tile_pool(name="lhs", bufs=3) as lpool, \
             tc.tile_pool(name="zt", bufs=3) as zpool, \
             tc.tile_pool(name="work", bufs=4) as wpool, \
             tc.tile_pool(name="ps_x", bufs=2, space="PSUM") as ps_x, \
             tc.tile_pool(name="ps_g", bufs=4, space="PSUM") as ps_g:

            identb = cpool.tile([128, 128], bf16)
            make_identity(nc, identb[:])
            z512 = cpool.tile([128, 4 * C], bf16)
            nc.vector.memset(z512[:], 0.0)
            ix = cpool.tile([128, n_cols * 8], i16)
            for rep in range(8):
                nc.sync.dma_start(out=ix[rep * 16:(rep + 1) * 16, :],
                                  in_=gidx[:])
            dinv_sb = cpool.tile([128, NGRP], f32)
            nc.sync.dma_start(out=dinv_sb[:], in_=dinv[:])
            w_sb, bb_sb = [], []
            for l in range(3):
                w = cpool.tile([128, C], bf16, name=f"w_sb{l}")
                nc.sync.dma_start(out=w[:], in_=w_in[l][:])
                w_sb.append(w)
                b = cpool.tile([128, couts[l]], f32, name=f"bb_sb{l}")
                nc.sync.dma_start(out=b[:], in_=bb_in[l][:])
                bb_sb.append(b)

            H = hpool.tile([128, NGRP * C], bf16)

            for lay in range(3):
                co = couts[lay]
                # ---- GEMM phase: Z = (H @ W) * dinv_src ----
                for g in range(NGRP):
                    if lay == 0:
                        ht = lpool.tile([128, 128], bf16, name="ht")
                        nc.sync.dma_start(
                            out=ht[:], in_=xT_d[:, g * 128:(g + 1) * 128])
                    else:
                        pst = ps_x.tile([128, 128], bf16, name="pst")
                        nc.tensor.transpose(
                            out=pst[:], in_=H[:, g * C:(g + 1) * C],
                            identity=identb[:])
                        ht = lpool.tile([128, 128], bf16, name="ht")
                        nc.vector.tensor_copy(out=ht[:], in_=pst[:])
                    psz = ps_x.tile([128, C], f32, name="psz")
                    nc.tensor.matmul(out=psz[:], lhsT=ht[:],
                                     rhs=w_sb[lay][:], start=True, stop=True)
                    zt = zpool.tile([128, C], bf16, name="zt")
                    nc.vector.tensor_scalar_mul(
                        out=zt[:], in0=psz[:], scalar1=dinv_sb[:, g:g + 1])
                    nc.sync.dma_start(out=zs[g * 128:(g + 1) * 128, :],
                                      in_=zt[:])

                nc.gpsimd.collective_compute(
                    "AllGather", mybir.AluOpType.bypass,
                    replica_groups=[list(range(NCORES))],
                    ins=[zs[:, :]], outs=[zf[lay][:, :]])

                # ---- aggregation phase ----
                # dma_gather is capped at 1024 indices (Q7 scratch), so
                # each (block, class) run is tiled into <=8-column gathers.
                TG = 8
                for blkg, runs in binfo:
                    psgs = {}
                    for g in blkg:
                        psgs[g] = ps_g.tile([128, 4 * C], f32, name="psg")
                        nc.tensor.matmul(
                            out=psgs[g][:], lhsT=identb[:], rhs=z512[:],
                            start=True, stop=False, skip_group_check=True)
                    totals = {g: sum(cmax[g]) for g in blkg}
                    for m in range(4):
                        c0, cw = runs[m]
                        for st in range(c0, c0 + cw, TG):
                            wt = min(TG, c0 + cw - st)
                            gs = gpool.tile([128, TG * C], bf16, name="gs")
                            nc.gpsimd.dma_gather(
                                gs[:, :wt * C].rearrange(
                                    "p (t e) -> p t e", e=C),
                                zf[lay][m::4, :],
                                ix[:, st * 8:(st + wt) * 8],
                                wt * 128, wt * 128, C, elem_step=512)
                            for g in blkg:
                                a = col_off[g][m]
                                b = a + cmax[g][m]
                                lo = max(a, st)
                                hi = min(b, st + wt)
                                if lo >= hi:
                                    continue
                                j = sum(cmax[g][:m]) + (lo - a)
                                cur = lo
                                while cur < hi:
                                    w = min(4 - (j % 4), hi - cur)
                                    nc.tensor.matmul(
                                        out=psgs[g][:, (j % 4) * C:
                                                    ((j % 4) + w) * C],
                                        lhsT=identb[:],
                                        rhs=gs[:, (cur - st) * C:
                                               (cur - st + w) * C],
                                        start=False,
                                        stop=(j + w == totals[g]),
                                        skip_group_check=True)
                                    j += w
                                    cur += w
                    for g in blkg:
                        psg = psgs[g]
                        tmp = wpool.tile([128, C], f32, name="tmp")
                        nc.vector.tensor_copy(out=tmp[:], in_=psg[:, :C])
                        for q in range(1, 4):
                            nc.vector.tensor_add(
                                out=tmp[:], in0=tmp[:],
                                in1=psg[:, q * C:(q + 1) * C])
                        nc.vector.tensor_scalar_mul(
                            out=tmp[:, :co], in0=tmp[:, :co],
                            scalar1=dinv_sb[:, g:g + 1])
                        nc.vector.tensor_add(out=tmp[:, :co],
                                             in0=tmp[:, :co],
                                             in1=bb_sb[lay][:])
                        if lay < 2:
                            nc.vector.tensor_scalar_max(
                                out=H[:, g * C:(g + 1) * C],
                                in0=tmp[:], scalar1=0.0)
                        else:
                            mx = wpool.tile([128, 1], f32, name="mx")
                            nc.vector.tensor_reduce(
                                out=mx[:], in_=tmp[:, :co],
                                axis=mybir.AxisListType.X,
                                op=mybir.AluOpType.max)
                            nmx = wpool.tile([128, 1], f32, name="nmx")
                            nc.vector.tensor_scalar_mul(
                                out=nmx[:], in0=mx[:], scalar1=-1.0)
                            ex = wpool.tile([128, C], f32, name="ex")
                            ssum = wpool.tile([128, 1], f32, name="ssum")
                            nc.scalar.activation(
                                out=ex[:, :co], in_=tmp[:, :co],
                                func=mybir.ActivationFunctionType.Exp,
                                bias=nmx[:], scale=1.0, accum_out=ssum[:])
                            lse = wpool.tile([128, 1], f32, name="lse")
                            nc.scalar.activation(
                                out=lse[:], in_=ssum[:],
                                func=mybir.ActivationFunctionType.Ln)
                            tot = wpool.tile([128, 1], f32, name="tot")
                            nc.vector.tensor_add(out=tot[:], in0=lse[:],
                                                 in1=mx[:])
                            ot = wpool.tile([128, COUT], f32, name="ot")
                            nc.vector.tensor_scalar_sub(
                                out=ot[:], in0=tmp[:, :co], scalar1=tot[:])
                            nc.sync.dma_start(
                                out=out_d[g * 128:(g + 1) * 128, :],
                                in_=ot[:])

    nc.compile()
    return nc


def kernel(x, edge_index, W1, b1, W2, b2, W3, b3):
    from concourse.bass_utils import run_bass_kernel_spmd

    in_maps, meta, perm2 = _preprocess(
        x, edge_index, W1, b1, W2, b2, W3, b3)
    nc = _build(meta)
    res = run_bass_kernel_spmd(nc, in_maps, core_ids=list(range(NCORES)))
    blocks = [res.results[k]["out"][:NBLK] for k in range(NCORES)]
    outp = np.concatenate(blocks, axis=0)
    out = np.empty((N, COUT), np.float32)
    out[perm2] = outp
    return out


# revision 11
# speedup vs baseline: 1.1290x; 1.1290x over previous
"""3-layer GCN forward (GCNConv x3 + log_softmax) on 8 Trainium2 cores.

v2: dma_gather-based aggregation (vs per-step indirect DMA).

Key ideas (shapes hardcoded for N=100000, Cin=Ch=128, Cout=47, 8 cores):
  A_hat = D^-1/2 (A+I) D^-1/2 fixed across layers:
      out = dinv_dst * segsum_dst( dinv_src * (H @ W) ) + b
  All feature math in fp16 (weights, Z exchange, gathers) with fp32 PSUM
  accumulation; rel-err budget is 2e-2.

  Aggregation uses nc.gpsimd.dma_gather (batched token gather): one
  instruction gathers up to ~6K rows. Its indices are int16 (<32768), so
  the 100352-row Z table is addressed via 4 residue classes mod 4:
  class m = rows with padded-global position % 4 == m, reached with
  elem_step=512 elements (1024B row stride) and base offset m*128 elems;
  idx = position // 4 <= 25087.

  Host side chooses the node permutation: degree-sorted round-robin deal
  across cores (aligns group degree profiles), then a greedy residue
  assignment balancing each destination row's in-messages across the 4
  classes (minimizes the per-(group,class) column padding), then nodes
  are placed within their 128-row group at slots == residue (mod 4).

  Column grid: per (group g, class m), cmax[g,m] = max over cores and
  over the group's 128 rows of the per-row class-m message count.
  Columns laid out block-major (blocks of 4 groups), class runs
  contiguous per (block, class) -> one dma_gather per (block, class).
  Accumulation: identity-matmul into a per-group PSUM bank with 4 lanes
  (psum [128, 4*128]), lane-reduced on DVE, then dinv_dst scale + bias
  + relu (or log_softmax on the last layer).

z row space: node (core k, local r) lives at padded-global row
k*12544 + r; rows [12500,12544) of each core block are zero (dinv=0
forces Z pads to 0); pad gather slots point at rows 12500+m (idx 3125).
"""
import numpy as np

NCORES = 8
N = 100000
NBLK = 12500
NPAD = 12544            # 98 * 128
NGRP = NPAD // 128      # 98
NALL = NCORES * NPAD    # 100352
C = 128
COUT = 47
BLK = 4                 # groups per gather block
ZPAD16 = (NCORES * NPAD // 8) and 3125  # (12500+m)//4 for m in 0..3


def _residue_greedy(src, dst, dstp, core_of, grp_of, d_g, rank):
    """Assign each node a residue class 0..3 balancing per-dst-row,
    per-class in-message counts against group thresholds ceil(d_g/4)."""
    order_src = np.argsort(src, kind="stable")
    dst_by_src = dstp[order_src]
    starts = np.zeros(N + 1, np.int64)
    cnt_src = np.bincount(src, minlength=N)
    np.cumsum(cnt_src, out=starts[1:])

    grp_of_pos = (np.arange(N) % NBLK) // 128
    T_pos = ((d_g + 3) // 4)[grp_of_pos].astype(np.int32)

    c = np.zeros((N, 4), np.int32)
    capq = np.full((NCORES, NGRP, 4), 32, np.int32)
    capq[:, NGRP - 1, :] = 21      # group 97 has 84 real rows
    res = np.full(N, -1, np.int8)
    BIG = 1 << 40
    for v in rank:
        s, e = starts[v], starts[v + 1]
        rows = dst_by_src[s:e]
        k, g = core_of[v], grp_of[v]
        cr = c[rows]
        over = cr + 1 - T_pos[rows][:, None]
        pen = np.where(over > 0, 1 << (4 * np.minimum(over, 9)), 0).astype(
            np.int64)
        sc = pen.sum(axis=0) + cr.sum(axis=0)
        sc = np.where(capq[k, g] > 0, sc, BIG)
        m = int(np.argmin(sc))
        res[v] = m
        capq[k, g, m] -= 1
        c[rows, m] += 1
    return res


def _preprocess(x, edge_index, W1, b1, W2, b2, W3, b3):
    import ml_dtypes
    bf = np.float16
    x = np.asarray(x, np.float32)
    ei = np.asarray(edge_index)
    loop = np.arange(N, dtype=np.int64)
    src = np.concatenate([ei[0], loop]).astype(np.int64)
    dst = np.concatenate([ei[1], loop]).astype(np.int64)

    deg = np.bincount(dst, minlength=N).astype(np.float32)
    dinv = 1.0 / np.sqrt(np.maximum(deg, 1.0))

    # degree-ranked round-robin deal across cores
    rank = np.argsort(-deg, kind="stable")
    perm = np.empty(N, np.int64)
    for k in range(NCORES):
        perm[k * NBLK:(k + 1) * NBLK] = rank[k::NCORES]
    inv = np.empty(N, np.int64)
    inv[perm] = np.arange(N)

    # unchunked per-group max degree (threshold source)
    dstp0 = inv[dst]
    cnt_tot = np.bincount(dstp0, minlength=N)
    cp = np.zeros((NCORES, NPAD), np.int64)
    cp[:, :NBLK] = cnt_tot.reshape(NCORES, NBLK)
    d_g = cp.reshape(NCORES, NGRP, 128).max(axis=2).max(axis=0)

    core_of = inv // NBLK
    grp_of = (inv % NBLK) // 128
    res = _residue_greedy(src, dst, dstp0, core_of, grp_of, d_g, rank)

    # re-place nodes within their group at slots == residue (mod 4)
    pos = np.arange(N)
    core_p = pos // NBLK
    r_p = pos % NBLK
    g_p = r_p // 128
    node_at = perm
    res_p = res[node_at].astype(np.int64)
    key2 = (core_p * NGRP + g_p) * 4 + res_p
    order2 = np.argsort(key2, kind="stable")
    cnt2 = np.bincount(key2, minlength=NCORES * NGRP * 4)
    cs2 = np.zeros(NCORES * NGRP * 4 + 1, np.int64)
    np.cumsum(cnt2, out=cs2[1:])
    jj = np.arange(N) - cs2[key2[order2]]
    slot = res_p[order2] + 4 * jj
    newpos = core_p[order2] * NBLK + g_p[order2] * 128 + slot
    perm2 = np.empty(N, np.int64)
    perm2[newpos] = node_at[order2]
    inv2 = np.empty(N, np.int64)
    inv2[perm2] = np.arange(N)

    # message coordinates under perm2
    srcp = inv2[src]
    dstp = inv2[dst]
    ksrc = srcp // NBLK
    srcg = ksrc * NPAD + (srcp - ksrc * NBLK)      # padded-global
    m_arr = (srcg % 4).astype(np.int64)
    idx16 = (srcg // 4).astype(np.int64)
    ecore = dstp // NBLK
    rloc = dstp - ecore * NBLK
    g_arr = rloc // 128

    # per (core,row,class) counts -> shared cmax[g,m]
    keyc = (ecore * NBLK + rloc) * 4 + m_arr
    cntc = np.bincount(keyc, minlength=NCORES * NBLK * 4)
    cpc = np.zeros((NCORES, NPAD, 4), np.int64)
    cpc[:, :NBLK, :] = cntc.reshape(NCORES, NBLK, 4)
    cmax = cpc.reshape(NCORES, NGRP, 128, 4).max(axis=2).max(axis=0)
    tot = cmax.sum(axis=1)
    bump = np.maximum(0, 4 - tot)
    cmax[:, 0] += bump                              # lanes need >= 4 cols

    # block-major class-run column layout
    blocks = [list(range(b, min(NGRP, b + BLK))) for b in range(0, NGRP, BLK)]
    col_off = np.zeros((NGRP, 4), np.int64)
    posc = 0
    binfo = []
    for blkg in blocks:
        runs = []
        for m in range(4):
            c0 = posc
            for g in blkg:
                col_off[g, m] = posc
                posc += cmax[g, m]
            runs.append((int(c0), int(posc - c0)))
        binfo.append((blkg, runs))
    n_cols = int(posc)
    cwmax = max(cw for _, runs in binfo for _, cw in runs)

    # gather tables [NCORES, 128, n_cols] int16
    order = np.lexsort((srcg, m_arr, rloc, ecore))
    e_s = ecore[order]
    r_s = rloc[order]
    m_s = m_arr[order]
    i_s = idx16[order]
    keys = (e_s * NBLK + r_s) * 4 + m_s
    css = np.zeros(NCORES * NBLK * 4 + 1, np.int64)
    np.cumsum(cntc, out=css[1:])
    j_s = np.arange(len(keys)) - css[keys]
    col_s = col_off[r_s // 128, m_s] + j_s
    tbl = np.full((NCORES, 128, n_cols), ZPAD16, np.int16)
    tbl[e_s, r_s % 128, col_s] = i_s.astype(np.int16)

    # wrap: flat i = col*128 + p lives at [i%16, i//16]
    idxarr = np.ascontiguousarray(
        tbl.transpose(0, 2, 1).reshape(NCORES, n_cols, 8, 16)
        .transpose(0, 3, 1, 2).reshape(NCORES, 16, n_cols * 8))

    dinv2 = dinv[perm2]
    dv = dinv2.reshape(NCORES, NBLK)
    dinv_loc = np.zeros((NCORES, 128, NGRP), np.float32)
    for k in range(NCORES):
        full = np.zeros(NPAD, np.float32)
        full[:NBLK] = dv[k]
        dinv_loc[k] = full.reshape(NGRP, 128).T

    xp = x[perm2]
    xT = np.zeros((NCORES, C, NPAD), bf)
    for k in range(NCORES):
        xT[k, :, :NBLK] = xp[k * NBLK:(k + 1) * NBLK].T.astype(bf)

    w3p = np.zeros((C, C), np.float32)
    w3p[:, :COUT] = np.asarray(W3, np.float32)
    Ws = [np.asarray(W1, np.float32).astype(bf),
          np.asarray(W2, np.float32).astype(bf),
          w3p.astype(bf)]
    bbs = [np.tile(np.asarray(b, np.float32)[None, :], (128, 1))
           for b in (b1, b2, b3)]

    in_maps = []
    for k in range(NCORES):
        in_maps.append({
            "xT": np.ascontiguousarray(xT[k]),
            "gidx": np.ascontiguousarray(idxarr[k]),
            "dinv": np.ascontiguousarray(dinv_loc[k]),
            "w1": Ws[0], "w2": Ws[1], "w3": Ws[2],
            "bb1": np.ascontiguousarray(bbs[0]),
            "bb2": np.ascontiguousarray(bbs[1]),
            "bb3": np.ascontiguousarray(bbs[2]),
        })
    meta = {
        "cmax": cmax.astype(int).tolist(),
        "col_off": col_off.astype(int).tolist(),
        "binfo": binfo,
        "n_cols": n_cols,
        "cwmax": int(cwmax),
    }
    return in_maps, meta, perm2


def _build(meta):
    from concourse import bacc, bass, mybir, tile
    from concourse.masks import make_identity
    from concourse import library_config
    f32 = mybir.dt.float32
    bf16 = mybir.dt.float16
    i16 = mybir.dt.int16
    cmax = meta["cmax"]
    col_off = meta["col_off"]
    binfo = meta["binfo"]
    n_cols = meta["n_cols"]
    couts = [C, C, COUT]

    nc = bacc.Bacc("TRN2", target_bir_lowering=False, debug=False,
                   num_devices=NCORES, num_swdge_queues=4)
    xT_d = nc.dram_tensor("xT", [C, NPAD], bf16, kind="ExternalInput")
    gidx = nc.dram_tensor("gidx", [16, n_cols * 8], i16,
                          kind="ExternalInput")
    dinv = nc.dram_tensor("dinv", [128, NGRP], f32, kind="ExternalInput")
    w_in = [nc.dram_tensor(f"w{l+1}", [C, C], bf16, kind="ExternalInput")
            for l in range(3)]
    bb_in = [nc.dram_tensor(f"bb{l+1}", [128, couts[l]], f32,
                            kind="ExternalInput") for l in range(3)]
    out_d = nc.dram_tensor("out", [NPAD, COUT], f32, kind="ExternalOutput")

    zs = nc.dram_tensor("zs", [NPAD, C], bf16)
    zf = [nc.dram_tensor(f"zf{l}", [NALL, C], bf16, addr_space="Shared")
          for l in range(3)]

    with tile.TileContext(nc) as tc:
        with tc.tile_pool(name="const", bufs=1) as cpool, \
             tc.tile_pool(name="hbuf", bufs=1) as hpool, \
             tc.tile_pool(name="gath", bufs=4) as gpool, \
             tc.tile_pool(name="lhs", bufs=3) as lpool, \
             tc.tile_pool(name="zt", bufs=3) as zpool, \
             tc.tile_pool(name="work", bufs=4) as wpool, \
             tc.tile_pool(name="ps_x", bufs=2, space="PSUM") as ps_x, \
             tc.tile_pool(name="ps_g", bufs=4, space="PSUM") as ps_g:

            identb = cpool.tile([128, 128], bf16)
            make_identity(nc, identb[:])
            z512 = cpool.tile([128, 4 * C], bf16)
            nc.vector.memset(z512[:], 0.0)
            ix = cpool.tile([128, n_cols * 8], i16)
            for rep in range(8):
                nc.sync.dma_start(out=ix[rep * 16:(rep + 1) * 16, :],
                                  in_=gidx[:])
            dinv_sb = cpool.tile([128, NGRP], f32)
            nc.sync.dma_start(out=dinv_sb[:], in_=dinv[:])
            w_sb, bb_sb = [], []
            for l in range(3):
                w = cpool.tile([128, C], bf16, name=f"w_sb{l}")
                nc.sync.dma_start(out=w[:], in_=w_in[l][:])
                w_sb.append(w)
                b = cpool.tile([128, couts[l]], f32, name=f"bb_sb{l}")
                nc.sync.dma_start(out=b[:], in_=bb_in[l][:])
                bb_sb.append(b)

            H = hpool.tile([128, NGRP * C], bf16)

            for lay in range(3):
                co = couts[lay]
                # ---- GEMM phase: Z = (H @ W) * dinv_src ----
                for g in range(NGRP):
                    if lay == 0:
                        ht = lpool.tile([128, 128], bf16, name="ht")
                        nc.sync.dma_start(
                            out=ht[:], in_=xT_d[:, g * 128:(g + 1) * 128])
                    else:
                        pst = ps_x.tile([128, 128], bf16, name="pst")
                        nc.tensor.transpose(
                            out=pst[:], in_=H[:, g * C:(g + 1) * C],
                            identity=identb[:])
                        ht = lpool.tile([128, 128], bf16, name="ht")
                        nc.vector.tensor_copy(out=ht[:], in_=pst[:])
                    psz = ps_x.tile([128, C], f32, name="psz")
                    nc.tensor.matmul(out=psz[:], lhsT=ht[:],
                                     rhs=w_sb[lay][:], start=True, stop=True)
                    zt = zpool.tile([128, C], bf16, name="zt")
                    nc.vector.tensor_scalar_mul(
                        out=zt[:], in0=psz[:], scalar1=dinv_sb[:, g:g + 1])
                    nc.sync.dma_start(out=zs[g * 128:(g + 1) * 128, :],
                                      in_=zt[:])

                nc.gpsimd.collective_compute(
                    "AllGather", mybir.AluOpType.bypass,
                    replica_groups=[list(range(NCORES))],
                    ins=[zs[:, :]], outs=[zf[lay][:, :]])

                # ---- aggregation phase ----
                # dma_gather is capped at 1024 indices (Q7 scratch), so
                # each (block, class) run is tiled into <=8-column gathers.
                # Spread across 4 SWDGE queues (separate Q7 cpu pairs).
                TG = 8
                qctr = 0
                for blkg, runs in binfo:
                    psgs = {}
                    for g in blkg:
                        psgs[g] = ps_g.tile([128, 4 * C], f32, name="psg")
                        nc.tensor.matmul(
                            out=psgs[g][:], lhsT=identb[:], rhs=z512[:],
                            start=True, stop=False, skip_group_check=True)
                    totals = {g: sum(cmax[g]) for g in blkg}
                    for m in range(4):
                        c0, cw = runs[m]
                        for st in range(c0, c0 + cw, TG):
                            wt = min(TG, c0 + cw - st)
                            gs = gpool.tile([128, TG * C], bf16, name="gs")
                            nc.gpsimd.dma_gather(
                                gs[:, :wt * C].rearrange(
                                    "p (t e) -> p t e", e=C),
                                zf[lay][m::4, :],
                                ix[:, st * 8:(st + wt) * 8],
                                wt * 128, wt * 128, C, elem_step=512,
                                queue_num=qctr % 4)
                            qctr += 1
                            for g in blkg:
                                a = col_off[g][m]
                                b = a + cmax[g][m]
                                lo = max(a, st)
                                hi = min(b, st + wt)
                                if lo >= hi:
                                    continue
                                j = sum(cmax[g][:m]) + (lo - a)
                                cur = lo
                                while cur < hi:
                                    w = min(4 - (j % 4), hi - cur)
                                    nc.tensor.matmul(
                                        out=psgs[g][:, (j % 4) * C:
                                                    ((j % 4) + w) * C],
                                        lhsT=identb[:],
                                        rhs=gs[:, (cur - st) * C:
                                               (cur - st + w) * C],
                                        start=False,
                                        stop=(j + w == totals[g]),
                                        skip_group_check=True)
                                    j += w
                                    cur += w
                    for g in blkg:
                        psg = psgs[g]
                        tmp = wpool.tile([128, C], f32, name="tmp")
                        nc.vector.tensor_copy(out=tmp[:], in_=psg[:, :C])
                        for q in range(1, 4):
                            nc.vector.tensor_add(
                                out=tmp[:], in0=tmp[:],
                                in1=psg[:, q * C:(q + 1) * C])
                        nc.vector.tensor_scalar_mul(
                            out=tmp[:, :co], in0=tmp[:, :co],
                            scalar1=dinv_sb[:, g:g + 1])
                        nc.vector.tensor_add(out=tmp[:, :co],
                                             in0=tmp[:, :co],
                                             in1=bb_sb[lay][:])
                        if lay < 2:
                            nc.vector.tensor_scalar_max(
                                out=H[:, g * C:(g + 1) * C],
                                in0=tmp[:], scalar1=0.0)
                        else:
                            mx = wpool.tile([128, 1], f32, name="mx")
                            nc.vector.tensor_reduce(
                                out=mx[:], in_=tmp[:, :co],
                                axis=mybir.AxisListType.X,
                                op=mybir.AluOpType.max)
                            nmx = wpool.tile([128, 1], f32, name="nmx")
                            nc.vector.tensor_scalar_mul(
                                out=nmx[:], in0=mx[:], scalar1=-1.0)
                            ex = wpool.tile([128, C], f32, name="ex")
                            ssum = wpool.tile([128, 1], f32, name="ssum")
                            nc.scalar.activation(
                                out=ex[:, :co], in_=tmp[:, :co],
                                func=mybir.ActivationFunctionType.Exp,
                                bias=nmx[:], scale=1.0, accum_out=ssum[:])
                            lse = wpool.tile([128, 1], f32, name="lse")
                            nc.scalar.activation(
                                out=lse[:], in_=ssum[:],
                                func=mybir.ActivationFunctionType.Ln)
                            tot = wpool.tile([128, 1], f32, name="tot")
                            nc.vector.tensor_add(out=tot[:], in0=lse[:],
                                                 in1=mx[:])
                            ot = wpool.tile([128, COUT], f32, name="ot")
                            nc.vector.tensor_scalar_sub(
                                out=ot[:], in0=tmp[:, :co], scalar1=tot[:])
                            nc.sync.dma_start(
                                out=out_d[g * 128:(g + 1) * 128, :],
                                in_=ot[:])

    nc.compile()
    return nc


def kernel(x, edge_index, W1, b1, W2, b2, W3, b3):
    from concourse.bass_utils import run_bass_kernel_spmd

    in_maps, meta, perm2 = _preprocess(
        x, edge_index, W1, b1, W2, b2, W3, b3)
    nc = _build(meta)
    res = run_bass_kernel_spmd(nc, in_maps, core_ids=list(range(NCORES)))
    blocks = [res.results[k]["out"][:NBLK] for k in range(NCORES)]
    outp = np.concatenate(blocks, axis=0)
    out = np.empty((N, COUT), np.float32)
    out[perm2] = outp
    return out


# revision 12
# speedup vs baseline: 1.7500x; 1.5500x over previous
"""3-layer GCN forward (GCNConv x3 + log_softmax) on 8 Trainium2 cores.

v2: dma_gather-based aggregation (vs per-step indirect DMA).

Key ideas (shapes hardcoded for N=100000, Cin=Ch=128, Cout=47, 8 cores):
  A_hat = D^-1/2 (A+I) D^-1/2 fixed across layers:
      out = dinv_dst * segsum_dst( dinv_src * (H @ W) ) + b
  All feature math in fp16 (weights, Z exchange, gathers) with fp32 PSUM
  accumulation; rel-err budget is 2e-2.

  Aggregation uses nc.gpsimd.dma_gather (batched token gather): one
  instruction gathers up to ~6K rows. Its indices are int16 (<32768), so
  the 100352-row Z table is addressed via 4 residue classes mod 4:
  class m = rows with padded-global position % 4 == m, reached with
  elem_step=512 elements (1024B row stride) and base offset m*128 elems;
  idx = position // 4 <= 25087.

  Host side chooses the node permutation: degree-sorted round-robin deal
  across cores (aligns group degree profiles), then a greedy residue
  assignment balancing each destination row's in-messages across the 4
  classes (minimizes the per-(group,class) column padding), then nodes
  are placed within their 128-row group at slots == residue (mod 4).

  Column grid: per (group g, class m), cmax[g,m] = max over cores and
  over the group's 128 rows of the per-row class-m message count.
  Columns laid out block-major (blocks of 4 groups), class runs
  contiguous per (block, class) -> one dma_gather per (block, class).
  Accumulation: identity-matmul into a per-group PSUM bank with 4 lanes
  (psum [128, 4*128]), lane-reduced on DVE, then dinv_dst scale + bias
  + relu (or log_softmax on the last layer).

z row space: node (core k, local r) lives at padded-global row
k*12544 + r; rows [12500,12544) of each core block are zero (dinv=0
forces Z pads to 0); pad gather slots point at rows 12500+m (idx 3125).
"""
import numpy as np

NCORES = 8
N = 100000
NBLK = 12500
NPAD = 12544            # 98 * 128
NGRP = NPAD // 128      # 98
NALL = NCORES * NPAD    # 100352
C = 128
COUT = 47
BLK = 4                 # groups per gather block
ZPAD16 = (NCORES * NPAD // 8) and 3125  # (12500+m)//4 for m in 0..3


def _residue_greedy(src, dst, dstp, core_of, grp_of, d_g, rank):
    """Assign each node a residue class 0..3 balancing per-dst-row,
    per-class in-message counts against group thresholds ceil(d_g/4)."""
    order_src = np.argsort(src, kind="stable")
    dst_by_src = dstp[order_src]
    starts = np.zeros(N + 1, np.int64)
    cnt_src = np.bincount(src, minlength=N)
    np.cumsum(cnt_src, out=starts[1:])

    grp_of_pos = (np.arange(N) % NBLK) // 128
    T_pos = ((d_g + 3) // 4)[grp_of_pos].astype(np.int32)

    c = np.zeros((N, 4), np.int32)
    capq = np.full((NCORES, NGRP, 4), 32, np.int32)
    capq[:, NGRP - 1, :] = 21      # group 97 has 84 real rows
    res = np.full(N, -1, np.int8)
    BIG = 1 << 40
    for v in rank:
        s, e = starts[v], starts[v + 1]
        rows = dst_by_src[s:e]
        k, g = core_of[v], grp_of[v]
        cr = c[rows]
        over = cr + 1 - T_pos[rows][:, None]
        pen = np.where(over > 0, 1 << (4 * np.minimum(over, 9)), 0).astype(
            np.int64)
        sc = pen.sum(axis=0) + cr.sum(axis=0)
        sc = np.where(capq[k, g] > 0, sc, BIG)
        m = int(np.argmin(sc))
        res[v] = m
        capq[k, g, m] -= 1
        c[rows, m] += 1
    return res


def _preprocess(x, edge_index, W1, b1, W2, b2, W3, b3):
    import ml_dtypes
    bf = ml_dtypes.bfloat16
    x = np.asarray(x, np.float32)
    ei = np.asarray(edge_index)
    loop = np.arange(N, dtype=np.int64)
    src = np.concatenate([ei[0], loop]).astype(np.int64)
    dst = np.concatenate([ei[1], loop]).astype(np.int64)

    deg = np.bincount(dst, minlength=N).astype(np.float32)
    dinv = 1.0 / np.sqrt(np.maximum(deg, 1.0))

    # degree-ranked round-robin deal across cores
    rank = np.argsort(-deg, kind="stable")
    perm = np.empty(N, np.int64)
    for k in range(NCORES):
        perm[k * NBLK:(k + 1) * NBLK] = rank[k::NCORES]
    inv = np.empty(N, np.int64)
    inv[perm] = np.arange(N)

    # unchunked per-group max degree (threshold source)
    dstp0 = inv[dst]
    cnt_tot = np.bincount(dstp0, minlength=N)
    cp = np.zeros((NCORES, NPAD), np.int64)
    cp[:, :NBLK] = cnt_tot.reshape(NCORES, NBLK)
    d_g = cp.reshape(NCORES, NGRP, 128).max(axis=2).max(axis=0)

    core_of = inv // NBLK
    grp_of = (inv % NBLK) // 128
    res = _residue_greedy(src, dst, dstp0, core_of, grp_of, d_g, rank)

    # re-place nodes within their group at slots == residue (mod 4)
    pos = np.arange(N)
    core_p = pos // NBLK
    r_p = pos % NBLK
    g_p = r_p // 128
    node_at = perm
    res_p = res[node_at].astype(np.int64)
    key2 = (core_p * NGRP + g_p) * 4 + res_p
    order2 = np.argsort(key2, kind="stable")
    cnt2 = np.bincount(key2, minlength=NCORES * NGRP * 4)
    cs2 = np.zeros(NCORES * NGRP * 4 + 1, np.int64)
    np.cumsum(cnt2, out=cs2[1:])
    jj = np.arange(N) - cs2[key2[order2]]
    slot = res_p[order2] + 4 * jj
    newpos = core_p[order2] * NBLK + g_p[order2] * 128 + slot
    perm2 = np.empty(N, np.int64)
    perm2[newpos] = node_at[order2]
    inv2 = np.empty(N, np.int64)
    inv2[perm2] = np.arange(N)

    # message coordinates under perm2
    srcp = inv2[src]
    dstp = inv2[dst]
    ksrc = srcp // NBLK
    srcg = ksrc * NPAD + (srcp - ksrc * NBLK)      # padded-global
    m_arr = (srcg % 4).astype(np.int64)
    idx16 = (srcg // 4).astype(np.int64)
    ecore = dstp // NBLK
    rloc = dstp - ecore * NBLK
    g_arr = rloc // 128

    # per (core,row,class) counts -> shared cmax[g,m]
    keyc = (ecore * NBLK + rloc) * 4 + m_arr
    cntc = np.bincount(keyc, minlength=NCORES * NBLK * 4)
    cpc = np.zeros((NCORES, NPAD, 4), np.int64)
    cpc[:, :NBLK, :] = cntc.reshape(NCORES, NBLK, 4)
    cmax = cpc.reshape(NCORES, NGRP, 128, 4).max(axis=2).max(axis=0)
    tot = cmax.sum(axis=1)
    bump = np.maximum(0, 4 - tot)
    cmax[:, 0] += bump                              # lanes need >= 4 cols

    # block-major class-run column layout
    blocks = [list(range(b, min(NGRP, b + BLK))) for b in range(0, NGRP, BLK)]
    col_off = np.zeros((NGRP, 4), np.int64)
    posc = 0
    binfo = []
    for blkg in blocks:
        runs = []
        for m in range(4):
            c0 = posc
            for g in blkg:
                col_off[g, m] = posc
                posc += cmax[g, m]
            runs.append((int(c0), int(posc - c0)))
        binfo.append((blkg, runs))
    n_cols = int(posc)
    cwmax = max(cw for _, runs in binfo for _, cw in runs)

    # gather tables [NCORES, 128, n_cols] int16
    order = np.lexsort((srcg, m_arr, rloc, ecore))
    e_s = ecore[order]
    r_s = rloc[order]
    m_s = m_arr[order]
    i_s = idx16[order]
    keys = (e_s * NBLK + r_s) * 4 + m_s
    css = np.zeros(NCORES * NBLK * 4 + 1, np.int64)
    np.cumsum(cntc, out=css[1:])
    j_s = np.arange(len(keys)) - css[keys]
    col_s = col_off[r_s // 128, m_s] + j_s
    tbl = np.full((NCORES, 128, n_cols), ZPAD16, np.int16)
    tbl[e_s, r_s % 128, col_s] = i_s.astype(np.int16)

    # wrap: flat i = col*128 + p lives at [i%16, i//16]
    idxarr = np.ascontiguousarray(
        tbl.transpose(0, 2, 1).reshape(NCORES, n_cols, 8, 16)
        .transpose(0, 3, 1, 2).reshape(NCORES, 16, n_cols * 8))

    dinv2 = dinv[perm2]
    dv = dinv2.reshape(NCORES, NBLK)
    dinv_loc = np.zeros((NCORES, 128, NGRP), np.float32)
    for k in range(NCORES):
        full = np.zeros(NPAD, np.float32)
        full[:NBLK] = dv[k]
        dinv_loc[k] = full.reshape(NGRP, 128).T

    xp = x[perm2]
    xT = np.zeros((NCORES, C, NPAD), bf)
    for k in range(NCORES):
        xT[k, :, :NBLK] = xp[k * NBLK:(k + 1) * NBLK].T.astype(bf)

    w3p = np.zeros((C, C), np.float32)
    w3p[:, :COUT] = np.asarray(W3, np.float32)
    Ws = [np.asarray(W1, np.float32).astype(bf),
          np.asarray(W2, np.float32).astype(bf),
          w3p.astype(bf)]
    bbs = [np.tile(np.asarray(b, np.float32)[None, :], (128, 1))
           for b in (b1, b2, b3)]

    in_maps = []
    for k in range(NCORES):
        in_maps.append({
            "xT": np.ascontiguousarray(xT[k]),
            "gidx": np.ascontiguousarray(idxarr[k]),
            "dinv": np.ascontiguousarray(dinv_loc[k]),
            "w1": Ws[0], "w2": Ws[1], "w3": Ws[2],
            "bb1": np.ascontiguousarray(bbs[0]),
            "bb2": np.ascontiguousarray(bbs[1]),
            "bb3": np.ascontiguousarray(bbs[2]),
        })
    meta = {
        "cmax": cmax.astype(int).tolist(),
        "col_off": col_off.astype(int).tolist(),
        "binfo": binfo,
        "n_cols": n_cols,
        "cwmax": int(cwmax),
    }
    return in_maps, meta, perm2


def _build(meta):
    from concourse import bacc, bass, mybir, tile
    from concourse.masks import make_identity
    from concourse import library_config
    f32 = mybir.dt.float32
    bf16 = mybir.dt.bfloat16
    i16 = mybir.dt.int16
    cmax = meta["cmax"]
    col_off = meta["col_off"]
    binfo = meta["binfo"]
    n_cols = meta["n_cols"]
    couts = [C, C, COUT]

    nc = bacc.Bacc("TRN2", target_bir_lowering=False, debug=False,
                   num_devices=NCORES, num_swdge_queues=4)
    xT_d = nc.dram_tensor("xT", [C, NPAD], bf16, kind="ExternalInput")
    gidx = nc.dram_tensor("gidx", [16, n_cols * 8], i16,
                          kind="ExternalInput")
    dinv = nc.dram_tensor("dinv", [128, NGRP], f32, kind="ExternalInput")
    w_in = [nc.dram_tensor(f"w{l+1}", [C, C], bf16, kind="ExternalInput")
            for l in range(3)]
    bb_in = [nc.dram_tensor(f"bb{l+1}", [128, couts[l]], f32,
                            kind="ExternalInput") for l in range(3)]
    out_d = nc.dram_tensor("out", [NPAD, COUT], f32, kind="ExternalOutput")

    zs = nc.dram_tensor("zs", [NPAD, C], bf16)
    zf = [nc.dram_tensor(f"zf{l}", [NALL, C], bf16, addr_space="Shared")
          for l in range(3)]

    with tile.TileContext(nc) as tc:
        with tc.tile_pool(name="const", bufs=1) as cpool, \
             tc.tile_pool(name="hbuf", bufs=1) as hpool, \
             tc.tile_pool(name="gath", bufs=8) as gpool, \
             tc.tile_pool(name="lhs", bufs=3) as lpool, \
             tc.tile_pool(name="zt", bufs=3) as zpool, \
             tc.tile_pool(name="work", bufs=4) as wpool, \
             tc.tile_pool(name="ps_x", bufs=1, space="PSUM") as ps_x, \
             tc.tile_pool(name="ps_g", bufs=6, space="PSUM") as ps_g:

            identb = cpool.tile([128, 128], bf16)
            make_identity(nc, identb[:])
            z512 = cpool.tile([128, 4 * C], bf16)
            nc.vector.memset(z512[:], 0.0)
            ix = cpool.tile([128, n_cols * 8], i16)
            for rep in range(8):
                nc.sync.dma_start(out=ix[rep * 16:(rep + 1) * 16, :],
                                  in_=gidx[:])
            dinv_sb = cpool.tile([128, NGRP], f32)
            nc.sync.dma_start(out=dinv_sb[:], in_=dinv[:])
            w_sb, bb_sb = [], []
            for l in range(3):
                w = cpool.tile([128, C], bf16, name=f"w_sb{l}")
                nc.sync.dma_start(out=w[:], in_=w_in[l][:])
                w_sb.append(w)
                b = cpool.tile([128, couts[l]], f32, name=f"bb_sb{l}")
                nc.sync.dma_start(out=b[:], in_=bb_in[l][:])
                bb_sb.append(b)

            H = hpool.tile([128, NGRP * C], bf16)

            for lay in range(3):
                co = couts[lay]
                # ---- GEMM phase: Z = (H @ W) * dinv_src ----
                for g in range(NGRP):
                    if lay == 0:
                        ht = lpool.tile([128, 128], bf16, name="ht")
                        nc.sync.dma_start(
                            out=ht[:], in_=xT_d[:, g * 128:(g + 1) * 128])
                    else:
                        pst = ps_x.tile([128, 128], bf16, name="pst")
                        nc.tensor.transpose(
                            out=pst[:], in_=H[:, g * C:(g + 1) * C],
                            identity=identb[:])
                        ht = lpool.tile([128, 128], bf16, name="ht")
                        nc.vector.tensor_copy(out=ht[:], in_=pst[:])
                    psz = ps_x.tile([128, C], f32, name="psz")
                    nc.tensor.matmul(out=psz[:], lhsT=ht[:],
                                     rhs=w_sb[lay][:], start=True, stop=True)
                    zt = zpool.tile([128, C], bf16, name="zt")
                    nc.vector.tensor_scalar_mul(
                        out=zt[:], in0=psz[:], scalar1=dinv_sb[:, g:g + 1])
                    nc.sync.dma_start(out=zs[g * 128:(g + 1) * 128, :],
                                      in_=zt[:])

                nc.gpsimd.collective_compute(
                    "AllGather", mybir.AluOpType.bypass,
                    replica_groups=[list(range(NCORES))],
                    ins=[zs[:, :]], outs=[zf[lay][:, :]])

                # ---- aggregation phase ----
                # dma_gather is capped at 1024 indices (Q7 scratch), so
                # each (block, class) run is tiled into <=8-column gathers.
                # Spread across 4 SWDGE queues (separate Q7 cpu pairs).
                TG = 8
                qctr = 0
                for blkg, runs in binfo:
                    psgs = {}
                    for g in blkg:
                        psgs[g] = ps_g.tile([128, 4 * C], f32, name="psg")
                        nc.tensor.matmul(
                            out=psgs[g][:], lhsT=identb[:], rhs=z512[:],
                            start=True, stop=False, skip_group_check=True)
                    totals = {g: sum(cmax[g]) for g in blkg}
                    for m in range(4):
                        c0, cw = runs[m]
                        for st in range(c0, c0 + cw, TG):
                            wt = min(TG, c0 + cw - st)
                            gs = gpool.tile([128, TG * C], bf16, name="gs")
                            nc.gpsimd.dma_gather(
                                gs[:, :wt * C].rearrange(
                                    "p (t e) -> p t e", e=C),
                                zf[lay][m::4, :],
                                ix[:, st * 8:(st + wt) * 8],
                                wt * 128, wt * 128, C, elem_step=512,
                                queue_num=qctr % 4)
                            qctr += 1
                            for g in blkg:
                                a = col_off[g][m]
                                b = a + cmax[g][m]
                                lo = max(a, st)
                                hi = min(b, st + wt)
                                if lo >= hi:
                                    continue
                                j = sum(cmax[g][:m]) + (lo - a)
                                cur = lo
                                while cur < hi:
                                    w = min(4 - (j % 4), hi - cur)
                                    nc.tensor.matmul(
                                        out=psgs[g][:, (j % 4) * C:
                                                    ((j % 4) + w) * C],
                                        lhsT=identb[:],
                                        rhs=gs[:, (cur - st) * C:
                                               (cur - st + w) * C],
                                        start=False,
                                        stop=(j + w == totals[g]),
                                        skip_group_check=True)
                                    j += w
                                    cur += w
                    for g in blkg:
                        psg = psgs[g]
                        tmp = wpool.tile([128, C], f32, name="tmp")
                        nc.vector.tensor_reduce(
                            out=tmp[:],
                            in_=psg[:].rearrange("p (q c) -> p c q", c=C),
                            axis=mybir.AxisListType.X,
                            op=mybir.AluOpType.add)
                        nc.vector.tensor_scalar_mul(
                            out=tmp[:, :co], in0=tmp[:, :co],
                            scalar1=dinv_sb[:, g:g + 1])
                        nc.vector.tensor_add(out=tmp[:, :co],
                                             in0=tmp[:, :co],
                                             in1=bb_sb[lay][:])
                        if lay < 2:
                            nc.vector.tensor_scalar_max(
                                out=H[:, g * C:(g + 1) * C],
                                in0=tmp[:], scalar1=0.0)
                        else:
                            mx = wpool.tile([128, 1], f32, name="mx")
                            nc.vector.tensor_reduce(
                                out=mx[:], in_=tmp[:, :co],
                                axis=mybir.AxisListType.X,
                                op=mybir.AluOpType.max)
                            nmx = wpool.tile([128, 1], f32, name="nmx")
                            nc.vector.tensor_scalar_mul(
                                out=nmx[:], in0=mx[:], scalar1=-1.0)
                            ex = wpool.tile([128, C], f32, name="ex")
                            ssum = wpool.tile([128, 1], f32, name="ssum")
                            nc.scalar.activation(
                                out=ex[:, :co], in_=tmp[:, :co],
                                func=mybir.ActivationFunctionType.Exp,
                                bias=nmx[:], scale=1.0, accum_out=ssum[:])
                            lse = wpool.tile([128, 1], f32, name="lse")
                            nc.scalar.activation(
                                out=lse[:], in_=ssum[:],
                                func=mybir.ActivationFunctionType.Ln)
                            tot = wpool.tile([128, 1], f32, name="tot")
                            nc.vector.tensor_add(out=tot[:], in0=lse[:],
                                                 in1=mx[:])
                            ot = wpool.tile([128, COUT], f32, name="ot")
                            nc.vector.tensor_scalar_sub(
                                out=ot[:], in0=tmp[:, :co], scalar1=tot[:])
                            nc.sync.dma_start(
                                out=out_d[g * 128:(g + 1) * 128, :],
                                in_=ot[:])

    nc.compile()
    return nc


def kernel(x, edge_index, W1, b1, W2, b2, W3, b3):
    from concourse.bass_utils import run_bass_kernel_spmd

    in_maps, meta, perm2 = _preprocess(
        x, edge_index, W1, b1, W2, b2, W3, b3)
    nc = _build(meta)
    res = run_bass_kernel_spmd(nc, in_maps, core_ids=list(range(NCORES)))
    blocks = [res.results[k]["out"][:NBLK] for k in range(NCORES)]
    outp = np.concatenate(blocks, axis=0)
    out = np.empty((N, COUT), np.float32)
    out[perm2] = outp
    return out
